# revision 7
# baseline (speedup 1.0000x reference)
"""ExpressionAttentionLayer Trainium2 kernel.

Algorithm (per reference):
  fused = concat(gene, expr) @ fused_W + fused_b
  q = (fused @ Wq + bq) * scale ; k = fused @ Wk + bk        (per-head slices)
  A = softmax(q k^T) ; A_bar = (A*M) / (sum_k |A*M| + eps)
  out = (A_bar @ V) @ out_W + out_b

Device decomposition: 8 cores; core d handles batch b = d//4 and head pair
(2*(d%4), 2*(d%4)+1). The fused+q/k projections are algebraically folded on
the host into a single [1024, 128] weight per core (Wqc = fused_W @ Wq_cols,
scale and biases folded), so each core computes q^T/k^T directly from
X^T = concat(gene,expr)^T via 8 accumulating matmuls.

On-chip, attention runs in the transposed layout S^T[key, query]:
  S^T tile = kT_h^T @ qT_h     (per 128-key tile x 512-query block)
  P = exp(S^T) * M^T           (no max-subtraction: logits are tiny by
                                construction, |logit| < ~1)
  numerator^T (and the key-sum denominator, via a ones-column appended to V)
  accumulate over key tiles:  num_h = V'_h^T @ P
  A_bar-normalisation happens after the V-contraction: num rows are scaled
  by broadcast(1/den) (a rank-1 ones x recip matmul), then projected through
  the per-head out_W rows. Host sums the 4 per-batch partials and adds out_b.

Matmuls use float32r (TF32-class, 1 cycle/row at N=512 vs 4 for fp32).
"""

import os
import sys
from contextlib import ExitStack

for _p in ("/opt/trn_rl_repo", "/root/.axon_site/_ro/trn_rl_repo"):
    if os.path.isdir(_p) and _p not in sys.path:
        sys.path.insert(0, _p)

import numpy as np

import concourse.bass as bass
import concourse.mybir as mybir
import concourse.tile as tile
from concourse import bacc
from concourse.bass_utils import run_bass_kernel_spmd

B, S, D, H, HD = 2, 2048, 512, 8, 64
KX = 2 * D            # fused-projection contraction dim
NCH = KX // 128       # 8 K-chunks
N_CORES = 8
QB = 512              # query block (one PSUM bank of fp32)
NQB = S // QB
KT = 128              # key tile
NKT = S // KT
NSUB = QB // 128
SCALE = 1.0 / np.sqrt(HD)

f32 = mybir.dt.float32
f32r = mybir.dt.float32r
bf16 = mybir.dt.bfloat16

# dtype of the mask stream and of the exp/mask product fed to the AV matmul.
# f32r keeps TF32-class accuracy; bf16 halves mask DMA and doubles the DVE
# multiply rate at ~1e-2 worst-case relative error.
M_DT = bf16 if os.environ.get("KERNEL_M_DT", "f32r") == "bf16" else f32r
P_DT = bf16 if os.environ.get("KERNEL_P_DT", "f32r") == "bf16" else f32r

_compiled = None
_last_results = None


def _build():
    nc = bacc.Bacc("TRN2", target_bir_lowering=False, debug=False,
                   num_devices=N_CORES)
    AF = mybir.ActivationFunctionType

    xt = nc.dram_tensor("xt", [128, NCH, S], f32r, kind="ExternalInput").ap()
    mt = nc.dram_tensor("mt", [S, S], M_DT, kind="ExternalInput").ap()
    wq = nc.dram_tensor("wq", [128, NCH, 128], f32r, kind="ExternalInput").ap()
    wk = nc.dram_tensor("wk", [128, NCH, 128], f32r, kind="ExternalInput").ap()
    bq = nc.dram_tensor("bq", [128, 1], f32, kind="ExternalInput").ap()
    bk = nc.dram_tensor("bk", [128, 1], f32, kind="ExternalInput").ap()
    v0 = nc.dram_tensor("v0", [128, NKT, HD + 1], P_DT, kind="ExternalInput").ap()
    v1 = nc.dram_tensor("v1", [128, NKT, HD + 1], P_DT, kind="ExternalInput").ap()
    wo0 = nc.dram_tensor("wo0", [HD, D], f32r, kind="ExternalInput").ap()
    wo1 = nc.dram_tensor("wo1", [HD, D], f32r, kind="ExternalInput").ap()
    ones = nc.dram_tensor("ones", [1, HD], f32r, kind="ExternalInput").ap()
    out = nc.dram_tensor("out", [S, D], f32, kind="ExternalOutput").ap()

    with tile.TileContext(nc) as tc:
        with tc.tile_pool(name="const", bufs=1) as const, \
             tc.tile_pool(name="mtp", bufs=6) as mtp, \
             tc.tile_pool(name="ep", bufs=3) as ep, \
             tc.tile_pool(name="pp", bufs=3) as pp, \
             tc.tile_pool(name="small", bufs=2) as small, \
             tc.tile_pool(name="outp", bufs=3) as outp:

            # ---- resident tensors -------------------------------------
            xt_s = const.tile([128, NCH, S], f32r)
            for ch in range(NCH):
                nc.sync.dma_start(out=xt_s[:, ch, :], in_=xt[:, ch, :])
            wq_s = const.tile([128, NCH, 128], f32r)
            wk_s = const.tile([128, NCH, 128], f32r)
            nc.sync.dma_start(out=wq_s, in_=wq)
            nc.sync.dma_start(out=wk_s, in_=wk)
            bq_s = const.tile([128, 1], f32)
            bk_s = const.tile([128, 1], f32)
            nc.sync.dma_start(out=bq_s, in_=bq)
            nc.sync.dma_start(out=bk_s, in_=bk)
            v0_s = const.tile([128, NKT, HD + 1], P_DT)
            v1_s = const.tile([128, NKT, HD + 1], P_DT)
            nc.sync.dma_start(out=v0_s, in_=v0)
            nc.sync.dma_start(out=v1_s, in_=v1)
            wo0_s = const.tile([HD, D], f32r)
            wo1_s = const.tile([HD, D], f32r)
            nc.sync.dma_start(out=wo0_s, in_=wo0)
            nc.sync.dma_start(out=wo1_s, in_=wo1)
            ones_s = const.tile([1, HD], f32r)
            nc.sync.dma_start(out=ones_s, in_=ones)

            qT = const.tile([128, S], f32r)
            kT = const.tile([128, S], f32r)

            # ---- phase A: q^T / k^T projections -----------------------
            # K-chunk-outer loop so each chunk's 8 matmuls overlap the DMA
            # of the next chunk; 8 live accumulation banks.
            with tc.tile_pool(name="ps_qk", bufs=8, space="PSUM") as ps_qk:
                acc = {}
                for t in range(2):
                    for g in range(NQB):
                        acc[(t, g)] = ps_qk.tile([128, QB], f32, tag="qk",
                                                 name=f"qk{t}_{g}")
                for ch in range(NCH):
                    for t, w_s in ((0, wq_s), (1, wk_s)):
                        for g in range(NQB):
                            nc.tensor.matmul(
                                acc[(t, g)],
                                w_s[:, ch, :],
                                xt_s[:, ch, g * QB:(g + 1) * QB],
                                start=(ch == 0), stop=(ch == NCH - 1))
                for t, b_s, dst in ((0, bq_s, qT), (1, bk_s, kT)):
                    for g in range(NQB):
                        nc.scalar.activation(
                            dst[:, g * QB:(g + 1) * QB], acc[(t, g)],
                            AF.Identity, bias=b_s)

            # ---- phase B: attention -----------------------------------
            ctxb = ExitStack()
            ps_st = ctxb.enter_context(
                tc.tile_pool(name="ps_st", bufs=2, space="PSUM"))
            ps_num = ctxb.enter_context(
                tc.tile_pool(name="ps_num", bufs=1, space="PSUM"))
            ps_sm = ctxb.enter_context(
                tc.tile_pool(name="ps_sm", bufs=2, space="PSUM"))
            for qb in range(NQB):
                q_lo, q_hi = qb * QB, (qb + 1) * QB
                num = [ps_num.tile([HD + 1, QB], f32, tag=f"num{h}",
                                   name=f"num{h}")
                       for h in range(2)]
                for kt in range(NKT):
                    k_lo, k_hi = kt * KT, (kt + 1) * KT
                    mt_t = mtp.tile([128, QB], M_DT, tag="mt")
                    nc.sync.dma_start(out=mt_t, in_=mt[k_lo:k_hi, q_lo:q_hi])
                    st = ps_st.tile([128, 2 * QB], f32, tag="st")
                    for h in range(2):
                        nc.tensor.matmul(
                            st[:, h * QB:(h + 1) * QB],
                            kT[h * HD:(h + 1) * HD, k_lo:k_hi],
                            qT[h * HD:(h + 1) * HD, q_lo:q_hi],
                            start=True, stop=True)
                    e_t = ep.tile([128, 2 * QB], P_DT, tag="e")
                    nc.scalar.activation(e_t, st, AF.Exp)
                    p_t = pp.tile([128, 2 * QB], P_DT, tag="p")
                    for h in range(2):
                        nc.vector.tensor_mul(
                            p_t[:, h * QB:(h + 1) * QB],
                            e_t[:, h * QB:(h + 1) * QB], mt_t)
                    for h, v_s in ((0, v0_s), (1, v1_s)):
                        nc.tensor.matmul(
                            num[h], v_s[:, kt, :],
                            p_t[:, h * QB:(h + 1) * QB],
                            start=(kt == 0), stop=(kt == NKT - 1))

                # epilogue: normalise + project
                sh = []
                for h in range(2):
                    rec32 = small.tile([1, QB], f32, tag=f"rec32_{h}")
                    nc.vector.reciprocal(rec32, num[h][HD:HD + 1, :])
                    rec = small.tile([1, QB], f32r, tag=f"rec_{h}")
                    nc.vector.tensor_copy(rec, rec32)
                    bc_ps = ps_sm.tile([128, D], f32, tag="sm")
                    nc.tensor.matmul(bc_ps[0:HD, :], ones_s, rec,
                                     start=True, stop=True)
                    bc_s = small.tile([HD, QB], f32, tag=f"bcs_{h}")
                    nc.vector.tensor_copy(bc_s, bc_ps[0:HD, :])
                    s_h = small.tile([HD, QB], f32r, tag=f"sh_{h}")
                    nc.vector.tensor_mul(s_h, num[h][0:HD, :], bc_s)
                    sh.append(s_h)
                for sub in range(NSUB):
                    pr = ps_sm.tile([128, D], f32, tag="sm")
                    nc.tensor.matmul(pr, sh[0][:, sub * 128:(sub + 1) * 128],
                                     wo0_s, start=True, stop=False)
                    nc.tensor.matmul(pr, sh[1][:, sub * 128:(sub + 1) * 128],
                                     wo1_s, start=False, stop=True)
                    o_t = outp.tile([128, D], f32, tag="o")
                    nc.scalar.copy(o_t, pr)
                    row = qb * QB + sub * 128
                    nc.sync.dma_start(out=out[row:row + 128, :], in_=o_t)
            ctxb.close()

    nc.compile()
    return nc


def _get_compiled():
    global _compiled
    if _compiled is None:
        _compiled = _build()
    return _compiled


def kernel(gene_emb, expr_emb, V, M, fused_W, fused_b, Wq, bq, Wk, bk,
           out_W, out_b):
    gene_emb = np.asarray(gene_emb, dtype=np.float32)
    expr_emb = np.asarray(expr_emb, dtype=np.float32)
    V = np.asarray(V, dtype=np.float32)
    M = np.asarray(M, dtype=np.float32)
    fused_W = np.asarray(fused_W, dtype=np.float32)
    fused_b = np.asarray(fused_b, dtype=np.float32)
    Wq_ = np.asarray(Wq, dtype=np.float32)
    bq_ = np.asarray(bq, dtype=np.float32)
    Wk_ = np.asarray(Wk, dtype=np.float32)
    bk_ = np.asarray(bk, dtype=np.float32)
    out_W = np.asarray(out_W, dtype=np.float32)
    out_b = np.asarray(out_b, dtype=np.float32)

    nc = _get_compiled()

    if M_DT == bf16 or P_DT == bf16:
        import ml_dtypes
    m_np = np.float32 if M_DT == f32r else ml_dtypes.bfloat16
    p_np = np.float32 if P_DT == f32r else ml_dtypes.bfloat16

    # fold fused projection into the q/k weights (float64 keeps the folding
    # itself out of the error budget)
    fW = fused_W.astype(np.float64)
    Wqc = (fW @ Wq_.astype(np.float64)) * SCALE
    bqc = (fused_b.astype(np.float64) @ Wq_.astype(np.float64) + bq_) * SCALE
    Wkc = fW @ Wk_.astype(np.float64)
    bkc = fused_b.astype(np.float64) @ Wk_.astype(np.float64) + bk_

    def chunk_major(a, nch):  # [nch*128, F] -> [128, nch, F]
        F = a.shape[1]
        return np.ascontiguousarray(
            a.reshape(nch, 128, F).transpose(1, 0, 2))

    xt_b, mt_b = [], []
    for b in range(B):
        XT = np.ascontiguousarray(
            np.concatenate([gene_emb[b], expr_emb[b]], axis=-1).T)
        xt_b.append(chunk_major(XT, NCH).astype(np.float32))
        mt_b.append(np.ascontiguousarray(M[b].T).astype(m_np))

    ones_col = np.ones((S, 1), np.float32)
    ones_row = np.ones((1, HD), np.float32)
    in_maps = []
    for d in range(N_CORES):
        b, p = d // 4, d % 4
        h0 = 2 * p
        cols = slice(p * 128, (p + 1) * 128)
        vs = []
        for h in (h0, h0 + 1):
            Vh = np.concatenate([V[b, :, h, :], ones_col], axis=1)  # [S,65]
            vs.append(chunk_major(Vh, NKT).astype(p_np))
        in_maps.append({
            "xt": xt_b[b],
            "mt": mt_b[b],
            "wq": chunk_major(Wqc[:, cols].astype(np.float32), NCH),
            "wk": chunk_major(Wkc[:, cols].astype(np.float32), NCH),
            "bq": bqc[cols].astype(np.float32).reshape(128, 1),
            "bk": bkc[cols].astype(np.float32).reshape(128, 1),
            "v0": vs[0],
            "v1": vs[1],
            "wo0": np.ascontiguousarray(out_W[h0 * HD:(h0 + 1) * HD, :]),
            "wo1": np.ascontiguousarray(out_W[(h0 + 1) * HD:(h0 + 2) * HD, :]),
            "ones": ones_row,
        })

    global _last_results
    res = run_bass_kernel_spmd(nc, in_maps, core_ids=list(range(N_CORES)))
    _last_results = res

    final = np.broadcast_to(out_b, (B, S, D)).astype(np.float32).copy()
    for d in range(N_CORES):
        final[d // 4] += res.results[d]["out"]
    return final


# revision 15
# speedup vs baseline: 1.1418x; 1.1418x over previous
"""ExpressionAttentionLayer Trainium2 kernel.

Algorithm (per reference):
  fused = concat(gene, expr) @ fused_W + fused_b
  q = (fused @ Wq + bq) * scale ; k = fused @ Wk + bk        (per-head slices)
  A = softmax(q k^T) ; A_bar = (A*M) / (sum_k |A*M| + eps)
  out = (A_bar @ V) @ out_W + out_b

Device decomposition: 8 cores; core d handles batch b = d//4 and head pair
(2*(d%4), 2*(d%4)+1). The fused+q/k projections are algebraically folded on
the host into a single [1024, 128] weight per core (Wqc = fused_W @ Wq_cols,
scale and biases folded), so each core computes q^T/k^T directly from
X^T = concat(gene,expr)^T via 8 accumulating matmuls.

On-chip, attention runs in the transposed layout S^T[key, query]:
  S^T tile = kT_h^T @ qT_h     (per 128-key tile x 512-query block)
  P = exp(S^T) * M^T           (no max-subtraction: logits are tiny by
                                construction, |logit| < ~1)
  numerator^T (and the key-sum denominator, via a ones-column appended to V)
  accumulate over key tiles:  num_h = V'_h^T @ P
  A_bar-normalisation happens after the V-contraction: num rows are scaled
  by broadcast(1/den) (a rank-1 ones x recip matmul), then projected through
  the per-head out_W rows. Host sums the 4 per-batch partials and adds out_b.

Matmuls use float32r (TF32-class, 1 cycle/row at N=512 vs 4 for fp32).
"""

import os
import sys
from contextlib import ExitStack

for _p in ("/opt/trn_rl_repo", "/root/.axon_site/_ro/trn_rl_repo"):
    if os.path.isdir(_p) and _p not in sys.path:
        sys.path.insert(0, _p)

import numpy as np

import concourse.bass as bass
import concourse.mybir as mybir
import concourse.tile as tile
from concourse import bacc
from concourse.bass_utils import run_bass_kernel_spmd

B, S, D, H, HD = 2, 2048, 512, 8, 64
KX = 2 * D            # fused-projection contraction dim
NCH = KX // 128       # 8 K-chunks
N_CORES = 8
QB = 512              # query block (one PSUM bank of fp32)
NQB = S // QB
KT = 128              # key tile
NKT = S // KT
NSUB = QB // 128
SCALE = 1.0 / np.sqrt(HD)

f32 = mybir.dt.float32
f32r = mybir.dt.float32r
bf16 = mybir.dt.bfloat16

# dtype of the mask stream and of the exp/mask product fed to the AV matmul.
# f32r keeps TF32-class accuracy; bf16 halves mask DMA and doubles the DVE
# multiply rate at ~1e-2 worst-case relative error.
M_DT = bf16 if os.environ.get("KERNEL_M_DT", "f32r") == "bf16" else f32r
P_DT = bf16 if os.environ.get("KERNEL_P_DT", "f32r") == "bf16" else f32r

_compiled = None
_last_results = None


def _build():
    nc = bacc.Bacc("TRN2", target_bir_lowering=False, debug=False,
                   num_devices=N_CORES)
    AF = mybir.ActivationFunctionType

    xt = nc.dram_tensor("xt", [128, NCH, S], f32r, kind="ExternalInput").ap()
    mt = nc.dram_tensor("mt", [S, S], M_DT, kind="ExternalInput").ap()
    wq = nc.dram_tensor("wq", [128, NCH, 128], f32r, kind="ExternalInput").ap()
    wk = nc.dram_tensor("wk", [128, NCH, 128], f32r, kind="ExternalInput").ap()
    bq = nc.dram_tensor("bq", [128, 1], f32, kind="ExternalInput").ap()
    bk = nc.dram_tensor("bk", [128, 1], f32, kind="ExternalInput").ap()
    v0 = nc.dram_tensor("v0", [128, NKT, HD + 1], P_DT, kind="ExternalInput").ap()
    v1 = nc.dram_tensor("v1", [128, NKT, HD + 1], P_DT, kind="ExternalInput").ap()
    wo0 = nc.dram_tensor("wo0", [HD, D], f32r, kind="ExternalInput").ap()
    wo1 = nc.dram_tensor("wo1", [HD, D], f32r, kind="ExternalInput").ap()
    ones = nc.dram_tensor("ones", [1, HD], f32r, kind="ExternalInput").ap()
    out = nc.dram_tensor("out", [S, D], f32, kind="ExternalOutput").ap()

    with tile.TileContext(nc) as tc:
        with tc.tile_pool(name="const", bufs=1) as const, \
             tc.tile_pool(name="mtp", bufs=8) as mtp, \
             tc.tile_pool(name="ep", bufs=4) as ep, \
             tc.tile_pool(name="pp", bufs=4) as pp, \
             tc.tile_pool(name="small", bufs=2) as small, \
             tc.tile_pool(name="outp", bufs=3) as outp:

            # ---- resident tensors -------------------------------------
            xt_s = const.tile([128, NCH, S], f32r)
            for ch in range(NCH):
                nc.sync.dma_start(out=xt_s[:, ch, :], in_=xt[:, ch, :])
            wq_s = const.tile([128, NCH, 128], f32r)
            wk_s = const.tile([128, NCH, 128], f32r)
            nc.sync.dma_start(out=wq_s, in_=wq)
            nc.sync.dma_start(out=wk_s, in_=wk)
            bq_s = const.tile([128, 1], f32)
            bk_s = const.tile([128, 1], f32)
            nc.sync.dma_start(out=bq_s, in_=bq)
            nc.sync.dma_start(out=bk_s, in_=bk)
            v0_s = const.tile([128, NKT, HD + 1], P_DT)
            v1_s = const.tile([128, NKT, HD + 1], P_DT)
            nc.sync.dma_start(out=v0_s, in_=v0)
            nc.sync.dma_start(out=v1_s, in_=v1)
            wo0_s = const.tile([HD, D], f32r)
            wo1_s = const.tile([HD, D], f32r)
            nc.sync.dma_start(out=wo0_s, in_=wo0)
            nc.sync.dma_start(out=wo1_s, in_=wo1)
            ones_s = const.tile([1, HD], f32r)
            nc.sync.dma_start(out=ones_s, in_=ones)

            qT = const.tile([128, S], f32r)
            kT = const.tile([128, S], f32r)

            # ---- phase A: q^T / k^T projections -----------------------
            # K-chunk-outer loop so each chunk's 8 matmuls overlap the DMA
            # of the next chunk; 8 live accumulation banks.
            with tc.tile_pool(name="ps_qk", bufs=8, space="PSUM") as ps_qk:
                acc = {}
                for t in range(2):
                    for g in range(NQB):
                        acc[(t, g)] = ps_qk.tile([128, QB], f32, tag="qk",
                                                 name=f"qk{t}_{g}")
                for ch in range(NCH):
                    for t, w_s in ((0, wq_s), (1, wk_s)):
                        for g in range(NQB):
                            nc.tensor.matmul(
                                acc[(t, g)],
                                w_s[:, ch, :],
                                xt_s[:, ch, g * QB:(g + 1) * QB],
                                start=(ch == 0), stop=(ch == NCH - 1))
                for t, b_s, dst in ((0, bq_s, qT), (1, bk_s, kT)):
                    for g in range(NQB):
                        nc.scalar.activation(
                            dst[:, g * QB:(g + 1) * QB], acc[(t, g)],
                            AF.Identity, bias=b_s)

            # ---- phase B: attention -----------------------------------
            ctxb = ExitStack()
            ps_st = ctxb.enter_context(
                tc.tile_pool(name="ps_st", bufs=2, space="PSUM"))
            ps_num = ctxb.enter_context(
                tc.tile_pool(name="ps_num", bufs=1, space="PSUM"))
            ps_epi = ctxb.enter_context(
                tc.tile_pool(name="ps_epi", bufs=2, space="PSUM"))
            for qb in range(NQB):
                q_lo, q_hi = qb * QB, (qb + 1) * QB
                num = [ps_num.tile([HD + 1, QB], f32, tag=f"num{h}",
                                   name=f"num{h}")
                       for h in range(2)]
                for kt in range(NKT):
                    k_lo, k_hi = kt * KT, (kt + 1) * KT
                    mt_t = mtp.tile([128, QB], M_DT, tag="mt")
                    nc.sync.dma_start(out=mt_t, in_=mt[k_lo:k_hi, q_lo:q_hi])
                    st = ps_st.tile([128, 2 * QB], f32, tag="st")
                    for h in range(2):
                        nc.tensor.matmul(
                            st[:, h * QB:(h + 1) * QB],
                            kT[h * HD:(h + 1) * HD, k_lo:k_hi],
                            qT[h * HD:(h + 1) * HD, q_lo:q_hi],
                            start=True, stop=True,
                            tile_position=(h * HD, 0))
                    e_t = ep.tile([128, 2 * QB], P_DT, tag="e")
                    nc.scalar.activation(e_t, st, AF.Exp)
                    p_t = pp.tile([128, 2 * QB], P_DT, tag="p")
                    for h in range(2):
                        nc.vector.tensor_mul(
                            p_t[:, h * QB:(h + 1) * QB],
                            e_t[:, h * QB:(h + 1) * QB], mt_t)
                    for h, v_s in ((0, v0_s), (1, v1_s)):
                        nc.tensor.matmul(
                            num[h], v_s[:, kt, :],
                            p_t[:, h * QB:(h + 1) * QB],
                            start=(kt == 0), stop=(kt == NKT - 1))

                # epilogue: normalise + project
                sh = []
                for h in range(2):
                    den = small.tile([1, QB], f32, tag=f"den_{h}")
                    nc.vector.tensor_copy(den, num[h][HD:HD + 1, :])
                    rec32 = small.tile([1, QB], f32, tag=f"rec32_{h}")
                    nc.vector.reciprocal_approx_fast(rec32, den)
                    rec = small.tile([1, QB], f32r, tag=f"rec_{h}")
                    nc.vector.tensor_copy(rec, rec32)
                    bc_ps = ps_epi.tile([128, D], f32, tag="epi",
                                        name=f"bc_ps{h}")
                    nc.tensor.matmul(bc_ps[0:HD, :], ones_s, rec,
                                     start=True, stop=True)
                    bc_s = small.tile([HD, QB], f32, tag=f"bcs_{h}")
                    nc.vector.tensor_copy(bc_s, bc_ps[0:HD, :])
                    s_h = small.tile([HD, QB], f32r, tag=f"sh_{h}")
                    nc.vector.tensor_mul(s_h, num[h][0:HD, :], bc_s)
                    sh.append(s_h)
                for sub in range(NSUB):
                    pr = ps_epi.tile([128, D], f32, tag="epi",
                                     name=f"pr{sub}")
                    nc.tensor.matmul(pr,
                                     sh[0][:, sub * 128:(sub + 1) * 128],
                                     wo0_s, start=True, stop=False)
                    nc.tensor.matmul(pr,
                                     sh[1][:, sub * 128:(sub + 1) * 128],
                                     wo1_s, start=False, stop=True)
                    o_t = outp.tile([128, D], f32, tag="o")
                    nc.vector.tensor_copy(o_t, pr)
                    row = qb * QB + sub * 128
                    nc.sync.dma_start(out=out[row:row + 128, :], in_=o_t)
            ctxb.close()

    nc.compile()
    return nc


def _get_compiled():
    global _compiled
    if _compiled is None:
        _compiled = _build()
    return _compiled


def kernel(gene_emb, expr_emb, V, M, fused_W, fused_b, Wq, bq, Wk, bk,
           out_W, out_b):
    gene_emb = np.asarray(gene_emb, dtype=np.float32)
    expr_emb = np.asarray(expr_emb, dtype=np.float32)
    V = np.asarray(V, dtype=np.float32)
    M = np.asarray(M, dtype=np.float32)
    fused_W = np.asarray(fused_W, dtype=np.float32)
    fused_b = np.asarray(fused_b, dtype=np.float32)
    Wq_ = np.asarray(Wq, dtype=np.float32)
    bq_ = np.asarray(bq, dtype=np.float32)
    Wk_ = np.asarray(Wk, dtype=np.float32)
    bk_ = np.asarray(bk, dtype=np.float32)
    out_W = np.asarray(out_W, dtype=np.float32)
    out_b = np.asarray(out_b, dtype=np.float32)

    nc = _get_compiled()

    if M_DT == bf16 or P_DT == bf16:
        import ml_dtypes
    m_np = np.float32 if M_DT == f32r else ml_dtypes.bfloat16
    p_np = np.float32 if P_DT == f32r else ml_dtypes.bfloat16

    # fold fused projection into the q/k weights (float64 keeps the folding
    # itself out of the error budget)
    fW = fused_W.astype(np.float64)
    Wqc = (fW @ Wq_.astype(np.float64)) * SCALE
    bqc = (fused_b.astype(np.float64) @ Wq_.astype(np.float64) + bq_) * SCALE
    Wkc = fW @ Wk_.astype(np.float64)
    bkc = fused_b.astype(np.float64) @ Wk_.astype(np.float64) + bk_

    def chunk_major(a, nch):  # [nch*128, F] -> [128, nch, F]
        F = a.shape[1]
        return np.ascontiguousarray(
            a.reshape(nch, 128, F).transpose(1, 0, 2))

    xt_b, mt_b = [], []
    for b in range(B):
        XT = np.ascontiguousarray(
            np.concatenate([gene_emb[b], expr_emb[b]], axis=-1).T)
        xt_b.append(chunk_major(XT, NCH).astype(np.float32))
        mt_b.append(np.ascontiguousarray(M[b].T).astype(m_np))

    ones_col = np.ones((S, 1), np.float32)
    ones_row = np.ones((1, HD), np.float32)
    in_maps = []
    for d in range(N_CORES):
        b, p = d // 4, d % 4
        h0 = 2 * p
        cols = slice(p * 128, (p + 1) * 128)
        vs = []
        for h in (h0, h0 + 1):
            Vh = np.concatenate([V[b, :, h, :], ones_col], axis=1)  # [S,65]
            vs.append(chunk_major(Vh, NKT).astype(p_np))
        in_maps.append({
            "xt": xt_b[b],
            "mt": mt_b[b],
            "wq": chunk_major(Wqc[:, cols].astype(np.float32), NCH),
            "wk": chunk_major(Wkc[:, cols].astype(np.float32), NCH),
            "bq": bqc[cols].astype(np.float32).reshape(128, 1),
            "bk": bkc[cols].astype(np.float32).reshape(128, 1),
            "v0": vs[0],
            "v1": vs[1],
            "wo0": np.ascontiguousarray(out_W[h0 * HD:(h0 + 1) * HD, :]),
            "wo1": np.ascontiguousarray(out_W[(h0 + 1) * HD:(h0 + 2) * HD, :]),
            "ones": ones_row,
        })

    global _last_results
    n_run = int(os.environ.get("KERNEL_CORES", N_CORES))
    if n_run < N_CORES:
        in_maps = in_maps[:1] * N_CORES  # timing experiment only
    res = run_bass_kernel_spmd(nc, in_maps[:n_run],
                               core_ids=list(range(n_run)))
    if n_run < N_CORES:
        res.results = list(res.results) * (N_CORES // n_run)
    _last_results = res

    final = np.broadcast_to(out_b, (B, S, D)).astype(np.float32).copy()
    for d in range(N_CORES):
        final[d // 4] += res.results[d]["out"]
    return final


# revision 35
# speedup vs baseline: 1.2591x; 1.1027x over previous
"""ExpressionAttentionLayer Trainium2 kernel.

Algorithm (per reference):
  fused = concat(gene, expr) @ fused_W + fused_b
  q = (fused @ Wq + bq) * scale ; k = fused @ Wk + bk        (per-head slices)
  A = softmax(q k^T) ; A_bar = (A*M) / (sum_k |A*M| + eps)
  out = (A_bar @ V) @ out_W + out_b

Device decomposition: 8 cores; core d handles batch b = d//4 and head pair
(2*(d%4), 2*(d%4)+1). The fused+q/k projections are algebraically folded on
the host into a single [1024, 128] weight per core (Wqc = fused_W @ Wq_cols,
scale and biases folded), so each core computes q^T/k^T directly from
X^T = concat(gene,expr)^T via 8 accumulating matmuls.

On-chip, attention runs in the transposed layout S^T[key, query]:
  S^T tile = kT_h^T @ qT_h     (per 128-key tile x 512-query block)
  P = exp(S^T) * M^T           (no max-subtraction: logits are tiny by
                                construction, |logit| < ~1)
  numerator^T (and the key-sum denominator, via a ones-column appended to V)
  accumulate over key tiles:  num_h = V'_h^T @ P
  A_bar-normalisation happens after the V-contraction: num rows are scaled
  by broadcast(1/den) (a rank-1 ones x recip matmul), then projected through
  the per-head out_W rows. Host sums the 4 per-batch partials and adds out_b.

Matmuls use float32r (TF32-class, 1 cycle/row at N=512 vs 4 for fp32).
"""

import os
import sys
from contextlib import ExitStack

for _p in ("/opt/trn_rl_repo", "/root/.axon_site/_ro/trn_rl_repo"):
    if os.path.isdir(_p) and _p not in sys.path:
        sys.path.insert(0, _p)

import numpy as np

import concourse.bass as bass
import concourse.mybir as mybir
import concourse.tile as tile
from concourse import bacc
from concourse.bass_utils import run_bass_kernel_spmd

B, S, D, H, HD = 2, 2048, 512, 8, 64
KX = 2 * D            # fused-projection contraction dim
NCH = KX // 128       # 8 K-chunks
N_CORES = 8
QB = 512              # query block (one PSUM bank of fp32)
NQB = S // QB
KT = 128              # key tile
NKT = S // KT
NSUB = QB // 128
SCALE = 1.0 / np.sqrt(HD)

f32 = mybir.dt.float32
f32r = mybir.dt.float32r
bf16 = mybir.dt.bfloat16

# dtype of the mask stream and of the exp/mask product fed to the AV matmul.
# f32r keeps TF32-class accuracy; bf16 halves mask DMA and doubles the DVE
# multiply rate at ~1e-2 worst-case relative error.
M_DT = bf16 if os.environ.get("KERNEL_M_DT", "f32r") == "bf16" else f32r
P_DT = bf16 if os.environ.get("KERNEL_P_DT", "f32r") == "bf16" else f32r
X_DT = bf16 if os.environ.get("KERNEL_X_DT", "f32r") == "bf16" else f32r

_compiled = None
_last_results = None


def _build():
    nc = bacc.Bacc("TRN2", target_bir_lowering=False, debug=False,
                   num_devices=N_CORES)
    AF = mybir.ActivationFunctionType

    xt = nc.dram_tensor("xt", [128, NCH, S], X_DT, kind="ExternalInput").ap()
    mt = nc.dram_tensor("mt", [S, S], M_DT, kind="ExternalInput").ap()
    wq = nc.dram_tensor("wq", [128, NCH, 128], X_DT, kind="ExternalInput").ap()
    wk = nc.dram_tensor("wk", [128, NCH, 128], X_DT, kind="ExternalInput").ap()
    bq = nc.dram_tensor("bq", [128, 1], f32, kind="ExternalInput").ap()
    bk = nc.dram_tensor("bk", [128, 1], f32, kind="ExternalInput").ap()
    v0 = nc.dram_tensor("v0", [128, NKT, HD + 1], P_DT, kind="ExternalInput").ap()
    v1 = nc.dram_tensor("v1", [128, NKT, HD + 1], P_DT, kind="ExternalInput").ap()
    wo0 = nc.dram_tensor("wo0", [HD, D], f32r, kind="ExternalInput").ap()
    wo1 = nc.dram_tensor("wo1", [HD, D], f32r, kind="ExternalInput").ap()
    ones = nc.dram_tensor("ones", [1, HD], f32r, kind="ExternalInput").ap()
    out = nc.dram_tensor("out", [S, D], f32, kind="ExternalOutput").ap()

    with tile.TileContext(nc) as tc:
        with tc.tile_pool(name="const", bufs=1) as const, \
             tc.tile_pool(name="mtp", bufs=6) as mtp, \
             tc.tile_pool(name="ep", bufs=4) as ep, \
             tc.tile_pool(name="pp", bufs=8) as pp, \
             tc.tile_pool(name="small", bufs=2) as small, \
             tc.tile_pool(name="outp", bufs=3) as outp:

            # ---- PE warm-up: dense matmul burst during the input DMA
            # (HAM un-throttles after ~3.4us of sustained PE activity)
            warm_in = const.tile([128, QB], bf16)
            nc.vector.memset(warm_in, 1.0)
            warm_o = const.tile([1, 8], f32)
            with tc.tile_pool(name="ps_warm", bufs=1, space="PSUM") as ps_w:
                warm_ps = ps_w.tile([128, QB], f32)
                for _ in range(20):
                    nc.tensor.matmul(warm_ps, warm_in[:, 0:128], warm_in,
                                     start=True, stop=True)
            # preload the Exp table while input DMAs run
            nc.scalar.activation(warm_o, warm_in[0:1, 0:8], AF.Exp)

            # ---- resident tensors -------------------------------------
            xt_s = const.tile([128, NCH, S], X_DT)
            for ch in range(NCH):
                nc.sync.dma_start(out=xt_s[:, ch, :], in_=xt[:, ch, :])
            wq_s = const.tile([128, NCH, 128], X_DT)
            wk_s = const.tile([128, NCH, 128], X_DT)
            nc.sync.dma_start(out=wq_s, in_=wq)
            nc.sync.dma_start(out=wk_s, in_=wk)
            bq_s = const.tile([128, 1], f32)
            bk_s = const.tile([128, 1], f32)
            nc.sync.dma_start(out=bq_s, in_=bq)
            nc.sync.dma_start(out=bk_s, in_=bk)
            v0_s = const.tile([128, NKT, HD + 1], P_DT)
            v1_s = const.tile([128, NKT, HD + 1], P_DT)
            nc.sync.dma_start(out=v0_s, in_=v0)
            nc.sync.dma_start(out=v1_s, in_=v1)
            wo0_s = const.tile([HD, D], f32r)
            wo1_s = const.tile([HD, D], f32r)
            nc.sync.dma_start(out=wo0_s, in_=wo0)
            nc.sync.dma_start(out=wo1_s, in_=wo1)
            ones_s = const.tile([1, HD], f32r)
            nc.sync.dma_start(out=ones_s, in_=ones)

            qT = const.tile([128, S], f32r)
            kT = const.tile([128, S], f32r)

            # ---- phase A: q^T / k^T projections -----------------------
            # K-chunk-outer loop so each chunk's 8 matmuls overlap the DMA
            # of the next chunk; 8 live accumulation banks.
            with tc.tile_pool(name="ps_qk", bufs=8, space="PSUM") as ps_qk:
                acc = {}
                for t in range(2):
                    for g in range(NQB):
                        acc[(t, g)] = ps_qk.tile([128, QB], f32, tag="qk",
                                                 name=f"qk{t}_{g}")
                for ch in range(NCH):
                    for t, w_s in ((0, wq_s), (1, wk_s)):
                        b_s, dst = ((bq_s, qT), (bk_s, kT))[t]
                        for g in range(NQB):
                            nc.tensor.matmul(
                                acc[(t, g)],
                                w_s[:, ch, :],
                                xt_s[:, ch, g * QB:(g + 1) * QB],
                                start=(ch == 0), stop=(ch == NCH - 1))
                            if ch == NCH - 1:
                                # bias-add as soon as each chain finishes so
                                # DVE overlaps the remaining matmuls
                                nc.vector.tensor_scalar_add(
                                    dst[:, g * QB:(g + 1) * QB],
                                    acc[(t, g)], b_s)

            # ---- phase B: attention -----------------------------------
            # Software-pipelined: AV matmuls lag the S^T/exp/mask chain by
            # LAG key-tiles so the PE instruction stream never blocks on the
            # ACT/DVE chain of the same tile; each query block's epilogue is
            # deferred into the next block's kt loop.
            LAG = 6
            ctxb = ExitStack()
            ps_st = ctxb.enter_context(
                tc.tile_pool(name="ps_st", bufs=2, space="PSUM"))
            ps_num = ctxb.enter_context(
                tc.tile_pool(name="ps_num", bufs=1, space="PSUM"))
            ps_epi = ctxb.enter_context(
                tc.tile_pool(name="ps_epi", bufs=2, space="PSUM"))

            def emit_st(qb, kt):
                q_lo, q_hi = qb * QB, (qb + 1) * QB
                k_lo, k_hi = kt * KT, (kt + 1) * KT
                mt_t = mtp.tile([128, QB], M_DT, tag="mt", name="mt_t")
                nc.sync.dma_start(out=mt_t, in_=mt[k_lo:k_hi, q_lo:q_hi])
                st = ps_st.tile([128, 2 * QB], f32, tag="st", name="st")
                for h in range(2):
                    nc.tensor.matmul(
                        st[:, h * QB:(h + 1) * QB],
                        kT[h * HD:(h + 1) * HD, k_lo:k_hi],
                        qT[h * HD:(h + 1) * HD, q_lo:q_hi],
                        start=True, stop=True,
                        tile_position=(h * HD, 0))
                e_t = ep.tile([128, 2 * QB], P_DT, tag="e", name="e_t")
                nc.scalar.activation(e_t, st, AF.Exp)
                p_t = pp.tile([128, 2 * QB], P_DT, tag="p", name="p_t")
                for h in range(2):
                    nc.vector.tensor_mul(
                        p_t[:, h * QB:(h + 1) * QB],
                        e_t[:, h * QB:(h + 1) * QB], mt_t)
                return p_t

            def emit_av(kt, num, p_t):
                for h, v_s in ((0, v0_s), (1, v1_s)):
                    nc.tensor.matmul(
                        num[h], v_s[:, kt, :],
                        p_t[:, h * QB:(h + 1) * QB],
                        start=(kt == 0), stop=(kt == NKT - 1))

            def make_epilogue(qb, num):
                state = {}

                def stage1():  # DVE-only: reciprocal chain
                    recs = []
                    for h in range(2):
                        den = small.tile([1, QB], f32, tag=f"den_{h}",
                                         name="den")
                        nc.vector.tensor_copy(den, num[h][HD:HD + 1, :])
                        rec32 = small.tile([1, QB], f32, tag=f"rec32_{h}",
                                           name="rec32")
                        nc.vector.reciprocal_approx_fast(rec32, den)
                        rec = small.tile([1, QB], f32r, tag=f"rec_{h}",
                                         name="rec")
                        nc.vector.tensor_copy(rec, rec32)
                        recs.append(rec)
                    state["recs"] = recs

                def make_scale(h):
                    def scale_h():  # broadcast matmul + scale numerator h
                        bc_ps = ps_epi.tile([128, D], f32, tag="epi",
                                            name=f"bc_ps{h}")
                        nc.tensor.matmul(bc_ps[0:HD, :], ones_s,
                                         state["recs"][h],
                                         start=True, stop=True)
                        bc_s = small.tile([HD, QB], f32, tag=f"bcs_{h}",
                                          name="bc_s")
                        nc.vector.tensor_copy(bc_s, bc_ps[0:HD, :])
                        s_h = small.tile([HD, QB], f32r, tag=f"sh_{h}",
                                         name="s_h")
                        nc.vector.tensor_mul(s_h, num[h][0:HD, :], bc_s)
                        state.setdefault("sh", {})[h] = s_h
                    return scale_h

                def make_proj(sub):
                    def proj_sub():  # output projection for one 128-q block
                        sh = state["sh"]
                        pr = ps_epi.tile([128, D], f32, tag="epi",
                                         name=f"pr{sub}")
                        nc.tensor.matmul(pr,
                                         sh[0][:, sub * 128:(sub + 1) * 128],
                                         wo0_s, start=True, stop=False)
                        nc.tensor.matmul(pr,
                                         sh[1][:, sub * 128:(sub + 1) * 128],
                                         wo1_s, start=False, stop=True)
                        o_t = outp.tile([128, D], f32, tag="o", name="o_t")
                        nc.vector.tensor_copy(o_t, pr)
                        row = qb * QB + sub * 128
                        nc.sync.dma_start(out=out[row:row + 128, :], in_=o_t)
                    return proj_sub

                return [stage1, make_scale(0), make_scale(1),
                        make_proj(0), make_proj(1), make_proj(2),
                        make_proj(3)]

            pending = []
            for qb in range(NQB):
                p_tiles = {}
                num = None
                stage_at = {2: 0, 4: 1, 5: 2, 8: 3, 10: 4, 12: 5, 14: 6}
                for kt in range(NKT):
                    p_tiles[kt] = emit_st(qb, kt)
                    if kt in stage_at and pending:
                        pending[stage_at[kt]]()
                    if kt == LAG:
                        num = [ps_num.tile([HD + 1, QB], f32, tag=f"num{h}",
                                           name=f"num{h}")
                               for h in range(2)]
                    if kt >= LAG:
                        emit_av(kt - LAG, num, p_tiles.pop(kt - LAG))
                for kt in range(NKT - LAG, NKT):
                    emit_av(kt, num, p_tiles.pop(kt))
                pending = make_epilogue(qb, num)
            for stage in pending:
                stage()
            ctxb.close()

    nc.compile()
    return nc


def _get_compiled():
    global _compiled
    if _compiled is None:
        _compiled = _build()
    return _compiled


def kernel(gene_emb, expr_emb, V, M, fused_W, fused_b, Wq, bq, Wk, bk,
           out_W, out_b):
    gene_emb = np.asarray(gene_emb, dtype=np.float32)
    expr_emb = np.asarray(expr_emb, dtype=np.float32)
    V = np.asarray(V, dtype=np.float32)
    M = np.asarray(M, dtype=np.float32)
    fused_W = np.asarray(fused_W, dtype=np.float32)
    fused_b = np.asarray(fused_b, dtype=np.float32)
    Wq_ = np.asarray(Wq, dtype=np.float32)
    bq_ = np.asarray(bq, dtype=np.float32)
    Wk_ = np.asarray(Wk, dtype=np.float32)
    bk_ = np.asarray(bk, dtype=np.float32)
    out_W = np.asarray(out_W, dtype=np.float32)
    out_b = np.asarray(out_b, dtype=np.float32)

    nc = _get_compiled()

    if bf16 in (M_DT, P_DT, X_DT):
        import ml_dtypes
    m_np = np.float32 if M_DT == f32r else ml_dtypes.bfloat16
    p_np = np.float32 if P_DT == f32r else ml_dtypes.bfloat16
    x_np = np.float32 if X_DT == f32r else ml_dtypes.bfloat16

    # fold fused projection into the q/k weights (float64 keeps the folding
    # itself out of the error budget)
    fW = fused_W.astype(np.float64)
    Wqc = (fW @ Wq_.astype(np.float64)) * SCALE
    bqc = (fused_b.astype(np.float64) @ Wq_.astype(np.float64) + bq_) * SCALE
    Wkc = fW @ Wk_.astype(np.float64)
    bkc = fused_b.astype(np.float64) @ Wk_.astype(np.float64) + bk_

    def chunk_major(a, nch):  # [nch*128, F] -> [128, nch, F]
        F = a.shape[1]
        return np.ascontiguousarray(
            a.reshape(nch, 128, F).transpose(1, 0, 2))

    xt_b, mt_b = [], []
    for b in range(B):
        XT = np.ascontiguousarray(
            np.concatenate([gene_emb[b], expr_emb[b]], axis=-1).T)
        xt_b.append(chunk_major(XT, NCH).astype(x_np))
        mt_b.append(np.ascontiguousarray(M[b].T).astype(m_np))

    ones_col = np.ones((S, 1), np.float32)
    ones_row = np.ones((1, HD), np.float32)
    in_maps = []
    for d in range(N_CORES):
        b, p = d // 4, d % 4
        h0 = 2 * p
        cols = slice(p * 128, (p + 1) * 128)
        vs = []
        for h in (h0, h0 + 1):
            Vh = np.concatenate([V[b, :, h, :], ones_col], axis=1)  # [S,65]
            vs.append(chunk_major(Vh, NKT).astype(p_np))
        in_maps.append({
            "xt": xt_b[b],
            "mt": mt_b[b],
            "wq": chunk_major(Wqc[:, cols].astype(np.float32),
                              NCH).astype(x_np),
            "wk": chunk_major(Wkc[:, cols].astype(np.float32),
                              NCH).astype(x_np),
            "bq": bqc[cols].astype(np.float32).reshape(128, 1),
            "bk": bkc[cols].astype(np.float32).reshape(128, 1),
            "v0": vs[0],
            "v1": vs[1],
            "wo0": np.ascontiguousarray(out_W[h0 * HD:(h0 + 1) * HD, :]),
            "wo1": np.ascontiguousarray(out_W[(h0 + 1) * HD:(h0 + 2) * HD, :]),
            "ones": ones_row,
        })

    global _last_results
    n_run = int(os.environ.get("KERNEL_CORES", N_CORES))
    if n_run < N_CORES:
        in_maps = in_maps[:1] * N_CORES  # timing experiment only
    res = run_bass_kernel_spmd(nc, in_maps[:n_run],
                               core_ids=list(range(n_run)))
    if n_run < N_CORES:
        res.results = list(res.results) * (N_CORES // n_run)
    _last_results = res

    final = np.broadcast_to(out_b, (B, S, D)).astype(np.float32).copy()
    for d in range(N_CORES):
        final[d // 4] += res.results[d]["out"]
    return final
